# revision 33
# baseline (speedup 1.0000x reference)
"""Causal self-attention with RoPE, fused Trainium2 Bass kernel, 8 NeuronCores.

Problem: x[4,2048,1024] @ W_attn[1024,3072] -> qkv; RoPE(q,k); causal
softmax attention (16 heads, d=64); y @ W_proj[1024,1024].

Sharding (data + head parallel): core c handles batch b=c//2 and heads
8*(c%2)..8*(c%2)+7.  W_attn is column-sharded by head, W_proj row-sharded;
each core emits a partial output projection and the host sums the two
partials per batch (the 2-way "all-reduce").

Kernel layout choices (per core):
 - Everything transposed: xT [D,T] in SBUF, q/k produced as qT/kT [d,T],
   attention computed as scoresT [k,q] so softmax-sum and PV contraction
   both run along the partition axis via matmuls (no transposes needed).
 - RoPE: head-dim channels are pre-permuted (via W_attn column permutation)
   into [e0..e15, o0..o15, e16..e31, o16..o31] per head so the rotate-half
   pairing is a 16<->16 swap inside each 32-partition quadrant, done with a
   single DVE stream_shuffle.  cos/sin tables (sign-folded) come from host.
 - No max-subtraction in softmax: scores/8 are ~N(0,0.4), exp is safe.
   l (row sum) comes free by appending a ones column to V (M=65 PV matmul).
 - fp16 operands everywhere on the PE (full-rate); fp32 accumulation.
 - Scores matmuls for a head pair run concurrently via row-tiled PE
   (stationaries at base partitions 0/64, separate PSUM banks).
"""

import sys

sys.path.insert(0, "/opt/trn_rl_repo")

import numpy as np

import concourse.bass as bass  # noqa: F401  (import registers engine classes)
import concourse.mybir as mybir
import concourse.tile as tile
from concourse import bacc
from concourse.bass_utils import run_bass_kernel_spmd

F16 = mybir.dt.float16
F32 = mybir.dt.float32

B, T, D = 4, 2048, 1024
N_HEAD, D_HEAD = 16, 64
ROPE_BASE = 10000.0
N_CORES = 8
HPC = N_HEAD // 2  # heads per core (8)
NPAIR = HPC // 2  # head pairs per core (4)
NKC = D // 128  # k-chunks (8)
NQC = T // 512  # q chunks of 512 (4)
NKB = T // 128  # k blocks of 128 (16)

SWAP_MASK = list(range(16, 32)) + list(range(0, 16))


def _build_program():
    nc = bacc.Bacc("TRN2", target_bir_lowering=False, debug=False,
                   num_devices=N_CORES)

    xT_d = nc.dram_tensor("xT", [D, T], F16, kind="ExternalInput").ap()
    wqk_d = nc.dram_tensor("wqk", [D, 1024], F16, kind="ExternalInput").ap()
    wv_d = nc.dram_tensor("wv", [D, 512], F16, kind="ExternalInput").ap()
    wp_d = nc.dram_tensor("wp", [512, D], F16, kind="ExternalInput").ap()
    cos_d = nc.dram_tensor("cos", [128, T], F16, kind="ExternalInput").ap()
    sin_d = nc.dram_tensor("sin", [128, T], F16, kind="ExternalInput").ap()
    mask_d = nc.dram_tensor("mask", [128, 128], F16, kind="ExternalInput").ap()
    outT_d = nc.dram_tensor("outT", [D, T], F32, kind="ExternalOutput").ap()

    with tile.TileContext(nc) as tc:
        with tc.tile_pool(name="const", bufs=1) as cpool, \
             tc.tile_pool(name="big", bufs=1) as big, \
             tc.tile_pool(name="rope", bufs=2) as rope, \
             tc.tile_pool(name="pbuf", bufs=8) as pbuf, \
             tc.tile_pool(name="rbuf", bufs=4) as rbuf, \
             tc.tile_pool(name="ost", bufs=3) as ost:

            # ---- constants & weights ----
            cos_sb = cpool.tile([128, T], F16)
            nc.sync.dma_start(cos_sb[:], cos_d)
            sin_sb = cpool.tile([128, T], F16)
            nc.sync.dma_start(sin_sb[:], sin_d)
            mask_sb = cpool.tile([128, 128], F16)
            nc.sync.dma_start(mask_sb[:], mask_d)
            ones16 = cpool.tile([1, 64], F16)
            nc.vector.memset(ones16[:], 1.0)

            xT_sb = big.tile([128, NKC, T], F16)
            wqk_sb = big.tile([128, NKC, 1024], F16)
            wv_sb = big.tile([128, NKC, 512], F16)
            for kc in range(NKC):
                nc.sync.dma_start(wv_sb[:, kc, :], wv_d[kc * 128:(kc + 1) * 128, :])
            # xT chunked by t so phase A-v can start on the first chunk;
            # wqk rides after the first chunk (needed later, by the aqk units)
            for tq in range(4):
                tsl = slice(tq * 512, (tq + 1) * 512)
                for kc in range(NKC):
                    nc.sync.dma_start(xT_sb[:, kc, tsl],
                                      xT_d[kc * 128:(kc + 1) * 128, tsl])
                if tq == 0:
                    for kc in range(NKC):
                        nc.sync.dma_start(wqk_sb[:, kc, :],
                                          wqk_d[kc * 128:(kc + 1) * 128, :])
            wp_sb = big.tile([128, NPAIR, 1024], F16)
            for cc in range(NPAIR):
                nc.sync.dma_start(wp_sb[:, cc, :], wp_d[cc * 128:(cc + 1) * 128, :])

            v_aug = big.tile([128, NKB, HPC, 65], F16)
            nc.vector.memset(v_aug[:], 1.0)

            qkT_sb = big.tile([128, 2 * NPAIR, T], F16)
            y_all = big.tile([128, NPAIR, T], F16)

            # ---- phase A: psum pools scoped so B/D pools fit in 8 banks ----
            phase_a = tc.tile_pool(name="vps", bufs=2, space="PSUM")
            vpsp = phase_a.__enter__()
            phase_a2 = tc.tile_pool(name="qkps", bufs=1, space="PSUM", side="right")
            qkpsp = phase_a2.__enter__()

            # ---- phase A-v: v in natural layout [t, d] per 128-row block ----
            for tt in range(NKB):
                vps_t = vpsp.tile([128, 512], F32, name=f"vps_{tt}", tag="vps")
                for kc in range(NKC):
                    nc.tensor.matmul(
                        vps_t[:],
                        lhsT=xT_sb[:, kc, tt * 128:(tt + 1) * 128],
                        rhs=wv_sb[:, kc, :],
                        start=(kc == 0), stop=(kc == NKC - 1),
                    )
                nc.vector.tensor_copy(
                    v_aug[:, tt, :, 0:64],
                    vps_t[:].rearrange("p (h d) -> p h d", h=HPC),
                )

            # ---- phase A-qk units: one (ctile, T-half) projection+RoPE ----
            # ctile i<4 holds q of head pair i; ctile 4+i holds k of pair i.
            # Units are emitted interleaved into phase B so the PE always has
            # dense matmul work while ACT crunches exps (keeps HAM at 8/8).
            def emit_aqk(ct, hf):
                qkps_t = qkpsp.tile([128, 1024], F32,
                                    name=f"qkps_{ct}_{hf}", tag="qkps")
                for tcc in range(2):
                    for kc in range(NKC):
                        nc.tensor.matmul(
                            qkps_t[:, tcc * 512:(tcc + 1) * 512],
                            lhsT=wqk_sb[:, kc, ct * 128:(ct + 1) * 128],
                            rhs=xT_sb[:, kc,
                                      hf * 1024 + tcc * 512:
                                      hf * 1024 + (tcc + 1) * 512],
                            start=(kc == 0), stop=(kc == NKC - 1),
                        )
                csl = slice(hf * 1024, (hf + 1) * 1024)
                xbf = rope.tile([128, 1024], F16, name=f"xbf_{ct}_{hf}", tag="xbf")
                nc.vector.tensor_copy(xbf[:], qkps_t[:])
                ybf = rope.tile([128, 1024], F16, name=f"ybf_{ct}_{hf}", tag="ybf")
                nc.vector.stream_shuffle(ybf[:], xbf[:], SWAP_MASK)
                t1 = rope.tile([128, 1024], F16, name=f"t1_{ct}_{hf}", tag="t1")
                nc.vector.tensor_tensor(t1[:], xbf[:], cos_sb[:, csl],
                                        mybir.AluOpType.mult)
                t2 = rope.tile([128, 1024], F16, name=f"t2_{ct}_{hf}", tag="t2")
                nc.vector.tensor_tensor(t2[:], ybf[:], sin_sb[:, csl],
                                        mybir.AluOpType.mult)
                nc.vector.tensor_add(qkT_sb[:, ct, csl], t1[:], t2[:])

            phase_a.__exit__(None, None, None)  # close vps pool

            phase_b = tc.tile_pool(name="sps", bufs=2, space="PSUM")
            spsp = phase_b.__enter__()
            phase_b2 = tc.tile_pool(name="ytps", bufs=2, space="PSUM")
            ytpsp = phase_b2.__enter__()

            emit_aqk(0, 0)
            emit_aqk(4, 0)

            opsp = None

            def emit_d_ot(qc, ot):
                ops_t = opsp.tile([128, 512], F32,
                                  name=f"ops_{qc}_{ot}", tag="ops")
                for pr in range(NPAIR):
                    nc.tensor.matmul(
                        ops_t[:],
                        lhsT=wp_sb[:, pr, ot * 128:(ot + 1) * 128],
                        rhs=y_all[:, pr, qc * 512:(qc + 1) * 512],
                        start=(pr == 0), stop=(pr == NPAIR - 1),
                    )
                st = ost.tile([128, 512], F32, name=f"st_{qc}_{ot}", tag="st")
                nc.scalar.copy(st[:], ops_t[:])
                nc.sync.dma_start(
                    outT_d[ot * 128:(ot + 1) * 128,
                           qc * 512:(qc + 1) * 512], st[:])

            dfill = []

            # ---- phase B: attention in scoresT layout, pair-outer ----
            for pr in range(NPAIR):
                q_t = qkT_sb[:, pr, :]
                k_t = qkT_sb[:, NPAIR + pr, :]
                for qc in range(NQC):
                    if pr == 3 and qc == 0:
                        # all aqk units done; swap PSUM pool for out-proj
                        phase_a2.__exit__(None, None, None)
                        phase_b3 = tc.tile_pool(name="ops", bufs=2, space="PSUM", side="right")
                        opsp = phase_b3.__enter__()
                    # A-qk units scheduled for this window (filler PE work).
                    # Pair 3's units ride in pair 2's windows so the aqk PSUM
                    # pool can close before pair 3 (its banks go to out-proj).
                    fill = {
                        (0, 0): [(0, 1)], (0, 1): [(4, 1)],
                        (0, 2): [(1, 0)], (0, 3): [(5, 0)],
                        (1, 0): [(1, 1)], (1, 1): [(5, 1)],
                        (1, 2): [(2, 0)], (1, 3): [(6, 0)],
                        (2, 0): [(2, 1)], (2, 1): [(6, 1)],
                        (2, 2): [(3, 0), (3, 1)],
                        (2, 3): [(7, 0), (7, 1)],
                    }.get((pr, qc), [])
                    yt0 = ytpsp.tile([65, 512], F32,
                                     name=f"yt0_{qc}_{pr}", tag="yt")
                    yt1 = ytpsp.tile([65, 512], F32,
                                     name=f"yt1_{qc}_{pr}", tag="yt")
                    yts = (yt0, yt1)
                    nkb = 4 * qc + 4
                    for kb in range(nkb):
                        if kb in (2, 6) and fill:
                            emit_aqk(*fill.pop(0))
                        if pr == 3 and dfill and kb % 2 == 1:
                            emit_d_ot(*dfill.pop(0))
                        off = max(0, (kb - 4 * qc) * 128)
                        sps_t = spsp.tile([128, 2, 512], F32,
                                          name=f"sps_{qc}_{pr}_{kb}", tag="sps")
                        for h in range(2):
                            nc.tensor.matmul(
                                sps_t[:, h, off:512],
                                lhsT=k_t[h * 64:(h + 1) * 64,
                                         kb * 128:(kb + 1) * 128],
                                rhs=q_t[h * 64:(h + 1) * 64,
                                        qc * 512 + off:(qc + 1) * 512],
                                start=True, stop=True,
                            )
                        pt = pbuf.tile([128, 2, 512], F16,
                                       name=f"pt_{qc}_{pr}_{kb}", tag="pt")
                        nc.scalar.activation(
                            pt[:, :, off:512], sps_t[:, :, off:512],
                            mybir.ActivationFunctionType.Exp, scale=0.125)
                        if kb >= 4 * qc:  # diagonal block: triangular mask
                            # one op for both heads: zero-step over the h dim
                            nc.gpsimd.affine_select(
                                out=pt[:, :, off:off + 128],
                                in_=pt[:, :, off:off + 128],
                                compare_op=mybir.AluOpType.is_ge,
                                fill=0.0, base=0,
                                pattern=[[0, 2], [1, 128]],
                                channel_multiplier=-1)
                        for h in range(2):
                            nc.tensor.matmul(
                                yts[h][:, off:512],
                                lhsT=v_aug[:, kb, 2 * pr + h, :],
                                rhs=pt[:, h, off:512],
                                start=(kb == 0), stop=(kb == nkb - 1),
                                skip_group_check=True,
                            )
                    # Normalize y/l off the PE critical path: copy y out of
                    # PSUM immediately (releases the yt bank), then
                    # 1/l -> f16 -> K=1 matmul broadcast into PSUM -> in-place
                    # multiply (one PSUM operand).
                    ysls = [y_all[h * 64:(h + 1) * 64, pr,
                                  qc * 512:(qc + 1) * 512] for h in range(2)]
                    # both yt-releasing copies FIRST so neither queues behind
                    # the other head's reciprocal chain on the DVE FIFO
                    for h in range(2):
                        nc.vector.tensor_copy(ysls[h], yts[h][0:64, :])
                    for h in range(2):
                        ysl = ysls[h]
                        lsb = rbuf.tile([1, 512], F32,
                                        name=f"lsb_{qc}_{pr}_{h}", tag="lsb")
                        nc.vector.tensor_copy(lsb[:], yts[h][64:65, :])
                        rr = rbuf.tile([1, 512], F32,
                                       name=f"rr_{qc}_{pr}_{h}", tag="rr")
                        nc.vector.reciprocal_approx_fast(rr[:], lsb[:])
                        rr16 = rbuf.tile([1, 512], F16,
                                         name=f"rr16_{qc}_{pr}_{h}", tag="rr16")
                        nc.scalar.copy(rr16[:], rr[:])
                        r64 = ytpsp.tile([64, 512], F32,
                                         name=f"r64_{qc}_{pr}_{h}", tag="yt")
                        nc.tensor.matmul(r64[:], lhsT=ones16[:], rhs=rr16[:],
                                         start=True, stop=True)
                        nc.vector.tensor_tensor(ysl, ysl, r64[:],
                                                mybir.AluOpType.mult)
                    if pr == 3:
                        dfill.extend((qc, ot) for ot in range(8))
            for qot in dfill:
                emit_d_ot(*qot)

            phase_b3.__exit__(None, None, None)
            phase_b2.__exit__(None, None, None)
            phase_b.__exit__(None, None, None)

    nc.compile()
    return nc


def _host_inputs(x, W_attn, W_proj):
    """Build the per-core input maps (host-side shard + layout prep)."""
    j = np.arange(16)
    perm = np.concatenate([2 * j, 2 * j + 1, 32 + 2 * j, 33 + 2 * j])

    # RoPE tables in the permuted-transposed layout, fp32 math then fp16.
    inv_freq = 1.0 / (ROPE_BASE ** (np.arange(0, D_HEAD, 2, dtype=np.float64)
                                    / D_HEAD))  # [32]
    t = np.arange(T, dtype=np.float64)
    freqs = np.outer(inv_freq, t)  # [32, T]
    jmap = np.concatenate([j, j, 16 + j, 16 + j])  # per-head 64 rows
    jmap = np.concatenate([jmap, jmap])  # 128 rows (2 heads)
    sign = np.tile(np.concatenate([-np.ones(16), np.ones(16)]), 4)  # [128]
    cos_tab = np.cos(freqs[jmap]).astype(np.float16)
    sin_tab = (sign[:, None] * np.sin(freqs[jmap])).astype(np.float16)

    mask = (np.arange(128)[:, None] <= np.arange(128)[None, :])
    mask = mask.astype(np.float16)

    in_maps = []
    for c in range(N_CORES):
        b, half = divmod(c, 2)
        heads = [8 * half + i for i in range(HPC)]
        # wqk: 4 q-pair ctiles then 4 k-pair ctiles, per-head perm'd cols
        cols = []
        for base in (0, D):  # q block, k block of W_attn
            for hp in range(NPAIR):
                for g in (heads[2 * hp], heads[2 * hp + 1]):
                    cols.append(base + g * D_HEAD + perm)
        wqk = W_attn[:, np.concatenate(cols)].astype(np.float16)
        wv = W_attn[:, 2 * D + 512 * half: 2 * D + 512 * (half + 1)] \
            .astype(np.float16)
        wp = W_proj[512 * half: 512 * (half + 1), :].astype(np.float16)
        xT = np.ascontiguousarray(x[b].T).astype(np.float16)
        in_maps.append({
            "xT": xT, "wqk": wqk, "wv": wv, "wp": wp,
            "cos": cos_tab, "sin": sin_tab, "mask": mask,
        })
    return in_maps


_NC_CACHE = None


def kernel(x, W_attn, W_proj, _trace=False):
    global _NC_CACHE
    x = np.asarray(x, dtype=np.float32)
    W_attn = np.asarray(W_attn, dtype=np.float32)
    W_proj = np.asarray(W_proj, dtype=np.float32)

    if _NC_CACHE is None:
        _NC_CACHE = _build_program()
    nc = _NC_CACHE

    in_maps = _host_inputs(x, W_attn, W_proj)
    res = run_bass_kernel_spmd(nc, in_maps, core_ids=list(range(N_CORES)),
                               trace=_trace)

    y = np.empty((B, T, D), dtype=np.float32)
    for b in range(B):
        y[b] = (res.results[2 * b]["outT"] + res.results[2 * b + 1]["outT"]).T
    if _trace:
        return y, res
    return y
